# revision 31
# baseline (speedup 1.0000x reference)
"""MoE adapter (top-1 of 4 experts, dense all-expert reference) on 8 TRN2 NeuronCores.

Strategy
--------
Data-parallel over the 32768 tokens (4096 per core); expert weights replicated.

The reference computes every expert's bottleneck MLP (D=768 -> H=192 -> D=768)
on all tokens and combines with the one-hot top-1 dispatch mask.  Since
4 experts x H=192 = 768, the four expert MLPs stack into two dense 768x768
matmuls:

    h_all = gelu(x @ W1_stacked + b1_stacked)       # [T, 768]
    mh    = h_all * expand(one_hot)                  # zero non-selected blocks
    y     = mh @ W2_stacked + one_hot @ b2           # [T, 768]
    out   = y + x

(the mask commutes with gelu because it is 0/1 valued).

On-chip layouts avoid all transposes:
  * host ships x three ways: token-major fp32 (skip add), and feature-major
    bf16 hi/lo split pair (matmul operand + exact-enough router),
  * mm1 produces h feature-major [H, tok] (lhsT = W1 chunk, rhs = x^T),
  * mm2 uses the masked h as the *stationary* operand so y comes out
    token-major [tok, D] and DMAs out contiguously.
  * router logits use the full (x_hi+x_lo) @ (rw_hi+rw_lo) cross product in
    fp32 PSUM accumulation -> bitwise-faithful argmax vs the fp32 reference.
"""

import numpy as np
import ml_dtypes

import concourse.bass as bass
import concourse.mybir as mybir
import concourse.tile as tile
from concourse import bacc
from concourse.bass_utils import run_bass_kernel_spmd

BF16 = ml_dtypes.bfloat16
F32 = np.float32

B, S, D = 16, 2048, 768
H, E = 192, 4
N_CORES = 8
TOK_TOTAL = B * S                 # 32768
TOK = TOK_TOTAL // N_CORES        # 4096 tokens per core
TILE = 512                        # tokens per pipeline tile
N_TILES = TOK // TILE             # 8
SUBT = TILE // 128                # 4 token subtiles of 128
KC = D // 128                     # 6 contraction chunks

_NC_CACHE = None


def _build_bass():
    dt = mybir.dt
    nc = bacc.Bacc("TRN2", target_bir_lowering=False)

    x32 = nc.dram_tensor("x32", [TOK, D], dt.float32, kind="ExternalInput")
    xht = nc.dram_tensor("xht", [D, TOK], dt.bfloat16, kind="ExternalInput")
    xlt = nc.dram_tensor("xlt", [D, TOK], dt.bfloat16, kind="ExternalInput")
    w1s = nc.dram_tensor("w1s", [D, D], dt.bfloat16, kind="ExternalInput")
    w2s = nc.dram_tensor("w2s", [D, D], dt.bfloat16, kind="ExternalInput")
    rwhl = nc.dram_tensor("rwhl", [D, 8], dt.bfloat16, kind="ExternalInput")
    eexp = nc.dram_tensor("eexp", [E, 128], dt.bfloat16, kind="ExternalInput")
    b2s = nc.dram_tensor("b2s", [E, D], dt.bfloat16, kind="ExternalInput")
    b1r = nc.dram_tensor("b1r", [128, KC], dt.float32, kind="ExternalInput")
    rb8 = nc.dram_tensor("rb8", [8, 1], dt.float32, kind="ExternalInput")
    out = nc.dram_tensor("out", [TOK, D], dt.float32, kind="ExternalOutput")

    # feature-major x views: [128 partitions, chunk, token]
    xht_r = xht.rearrange("(c p) t -> p c t", p=128)
    xlt_r = xlt.rearrange("(c p) t -> p c t", p=128)

    add = mybir.AluOpType.add
    mult = mybir.AluOpType.mult
    amax = mybir.AluOpType.max
    iseq = mybir.AluOpType.is_equal

    with tile.TileContext(nc) as tc:
        with (
            tc.tile_pool(name="const", bufs=1) as const,
            tc.tile_pool(name="xin", bufs=3) as xin,
            tc.tile_pool(name="hbuf", bufs=2) as hbuf,
            tc.tile_pool(name="obuf", bufs=2) as obuf,
            tc.tile_pool(name="small", bufs=2) as small,
            tc.tile_pool(name="ps_r", bufs=1, space="PSUM") as ps_r,
            tc.tile_pool(name="ps_h", bufs=2, space="PSUM") as ps_h,
            tc.tile_pool(name="ps_m", bufs=1, space="PSUM") as ps_m,
            tc.tile_pool(name="ps_y", bufs=4, space="PSUM") as ps_y,
        ):
            # Small constants go on the gpsimd (SWDGE) queue; the two big weight
            # matrices ride the sync HWDGE FIFO interleaved with tile-0's loads
            # in exact first-use order (xh, xl, w1, x32, w2).
            rwsb = const.tile([128, KC, 8], dt.bfloat16)
            nc.gpsimd.dma_start(rwsb, rwhl.rearrange("(c p) e -> p c e", p=128))
            w1sb = const.tile([128, KC, D], dt.bfloat16)
            nc.gpsimd.dma_start(w1sb, w1s.rearrange("(c p) h -> p c h", p=128))
            rbsb = const.tile([8, 1], dt.float32)
            nc.gpsimd.dma_start(rbsb, rb8[:])
            b1sb = const.tile([128, KC], dt.float32)
            nc.gpsimd.dma_start(b1sb, b1r[:])
            eesb = const.tile([E, 128], dt.bfloat16)
            nc.gpsimd.dma_start(eesb, eexp[:])
            b2sb = const.tile([E, D], dt.bfloat16)
            nc.gpsimd.dma_start(b2sb, b2s[:])
            w2sb = const.tile([128, KC, D], dt.bfloat16)

            def load_tiles(it):
                t0 = it * TILE
                xh = xin.tile([128, KC, TILE], dt.bfloat16, tag="xh")
                nc.sync.dma_start(xh, xht_r[:, :, t0 : t0 + TILE])
                xl = xin.tile([128, KC, TILE], dt.bfloat16, tag="xl")
                nc.sync.dma_start(xl, xlt_r[:, :, t0 : t0 + TILE])
                x32t = xin.tile([128, SUBT, D], dt.float32, tag="x32t")
                nc.sync.dma_start(
                    x32t, x32[t0 : t0 + TILE].rearrange("(a p) d -> p a d", p=128)
                )
                if it == 0:
                    nc.sync.dma_start(
                        w2sb, w2s.rearrange("(c p) h -> p c h", p=128)
                    )
                return xh, xl, x32t

            def router_onehot(xh, xl):
                """logits^T in psum -> one-hot mask mt32[0:4] [4, TILE] bf16."""
                # rows 0:4 accumulate (x_hi + x_lo) @ rw_hi  (+rb via ACT bias)
                # rows 4:8 accumulate (x_hi + x_lo) @ rw_lo
                psr = ps_r.tile([8, TILE], dt.float32, tag="psr")
                for kc in range(KC):
                    nc.tensor.matmul(
                        psr, rwsb[:, kc, :], xh[:, kc, :],
                        start=(kc == 0), stop=False,
                    )
                for kc in range(KC):
                    nc.tensor.matmul(
                        psr, rwsb[:, kc, :], xl[:, kc, :],
                        start=False, stop=(kc == KC - 1),
                    )
                # alignment-safe one-hot argmax via DVE 32x32 stream transpose
                lt32s = small.tile([32, TILE], dt.float32, tag="lt32s")
                nc.scalar.activation(
                    lt32s[0:8], psr,
                    mybir.ActivationFunctionType.Identity,
                    bias=rbsb, scale=1.0,
                )
                # token-major blocks: lt32[p, 32g+r] = lt32s[r, 32g+p]
                lt32 = small.tile([32, TILE], dt.float32, tag="lt32")
                nc.vector.transpose(lt32, lt32s)
                v = lt32.rearrange("p (g r) -> p g r", r=32)
                lt_tok = small.tile([32, TILE // 32, E], dt.float32, tag="lt_tok")
                nc.vector.tensor_tensor(lt_tok, v[:, :, 0:4], v[:, :, 4:8], add)
                mxg = small.tile([32, TILE // 32], dt.float32, tag="mxg")
                nc.vector.tensor_reduce(
                    out=mxg, in_=lt_tok, axis=mybir.AxisListType.X, op=amax
                )
                mtb = small.tile([32, TILE], dt.bfloat16, tag="mtb")
                mview = mtb.rearrange("p (g r) -> p g r", r=32)
                nc.vector.tensor_tensor(
                    mview[:, :, 0:4], lt_tok,
                    mxg[:, :, None].to_broadcast((32, TILE // 32, E)), iseq,
                )
                # back-transpose: mt32[e, t] = one_hot[t, e] for e < 4
                mt32 = small.tile([32, TILE], dt.bfloat16, tag="mt32")
                nc.vector.transpose(mt32, mtb)
                return mt32

            # ---- PE warm-up burst: spin the HAM up to K=8/8 during the DMA head
            dummy = const.tile([128, TILE], dt.bfloat16)
            nc.vector.memset(dummy, 0.0)
            psd = ps_h.tile([128, TILE], dt.float32, tag="psh")
            for _ in range(10):
                nc.tensor.matmul(psd, dummy[:, 0:128], dummy, start=True, stop=True)

            # software pipeline: router/one-hot for tile n+1 issues at the end of
            # iteration n, so the mask chain latency hides under mm1/mm2.
            # Loads run two tiles ahead.
            tiles = {0: load_tiles(0)}
            mt32 = router_onehot(tiles[0][0], tiles[0][1])
            tiles[1] = load_tiles(1)

            for it in range(N_TILES):
                t0 = it * TILE
                mt = mt32[0:4]
                xh, xl, x32t = tiles[it]

                if it + 2 < N_TILES:
                    tiles[it + 2] = load_tiles(it + 2)

                # ---- mm1: h^T = gelu(W1^T x + b1), then mask ----
                # experts are interleaved along H (unit j of expert e at 4j+e),
                # so the expanded one-hot is the same [128, TILE] tile for every
                # H-chunk: a single K=4 matmul per tile.
                psm = ps_m.tile([128, TILE], dt.float32, tag="psm")
                nc.tensor.matmul(psm, eesb, mt, start=True, stop=True)
                mh = hbuf.tile([128, KC, TILE], dt.bfloat16, tag="mh")
                hchunk = hbuf.tile([128, KC, TILE], dt.bfloat16, tag="hchunk")
                for hc in range(KC):
                    psh = ps_h.tile([128, TILE], dt.float32, tag="psh")
                    for kc in range(KC):
                        nc.tensor.matmul(
                            psh,
                            w1sb[:, kc, hc * 128 : (hc + 1) * 128],
                            xh[:, kc, :],
                            start=(kc == 0), stop=(kc == KC - 1),
                        )
                    nc.scalar.activation(
                        hchunk[:, hc, :], psh,
                        mybir.ActivationFunctionType.Gelu,
                        bias=b1sb[:, hc : hc + 1], scale=1.0,
                    )
                    nc.vector.tensor_tensor(
                        mh[:, hc, :], hchunk[:, hc, :], psm, mult
                    )

                # ---- mm2: y = mh^T.T @ W2 + one_hot @ b2, token-major ----
                out_r = out[t0 : t0 + TILE].rearrange("(a p) d -> p a d", p=128)
                for a in range(SUBT):
                    osb = obuf.tile([128, D], dt.float32, tag="osb")
                    for half in range(2):
                        d0 = half * 384
                        psy = ps_y.tile([128, 384], dt.float32, tag="psy")
                        for hc in range(KC):
                            nc.tensor.matmul(
                                psy,
                                mh[:, hc, a * 128 : (a + 1) * 128],
                                w2sb[:, hc, d0 : d0 + 384],
                                start=(hc == 0), stop=False,
                            )
                        nc.tensor.matmul(
                            psy,
                            mt[:, a * 128 : (a + 1) * 128],
                            b2sb[:, d0 : d0 + 384],
                            start=False, stop=True,
                        )
                        nc.vector.tensor_tensor(
                            osb[:, d0 : d0 + 384], psy,
                            x32t[:, a, d0 : d0 + 384], add,
                        )
                    # per-subtile store on the ACT HWDGE ring (doesn't block loads)
                    nc.scalar.dma_start(out_r[:, a, :], osb)

                if it + 1 < N_TILES:
                    mt32 = router_onehot(tiles[it + 1][0], tiles[it + 1][1])
                del tiles[it]

    nc.compile()
    return nc


def _prep_inputs(x, router_w, router_b, w1, b1, w2, b2):
    """Host-side packing: split/cast/transpose; returns per-core input dicts."""
    xf = np.ascontiguousarray(np.asarray(x, dtype=F32).reshape(TOK_TOTAL, D))
    x_hi = xf.astype(BF16)
    x_lo = (xf - x_hi.astype(F32)).astype(BF16)

    rw = np.asarray(router_w, dtype=F32)
    rw_hi = rw.astype(BF16)
    rw_lo = (rw - rw_hi.astype(F32)).astype(BF16)
    rwhl = np.ascontiguousarray(np.concatenate([rw_hi, rw_lo], axis=1))  # [D, 8]

    w1f = np.asarray(w1, dtype=F32)           # [E, D, H]
    w2f = np.asarray(w2, dtype=F32)           # [E, H, D]
    b1f = np.asarray(b1, dtype=F32)           # [E, H]
    b2f = np.asarray(b2, dtype=F32)           # [E, D]
    rb = np.asarray(router_b, dtype=F32)      # [E]

    # experts interleaved along the stacked hidden dim: unit j of expert e
    # lives at index 4j + e  -> the one-hot expansion pattern repeats every
    # 4 partitions, identically for each 128-row chunk.
    w1s = np.ascontiguousarray(w1f.transpose(1, 2, 0).reshape(D, H * E)).astype(BF16)
    w2s = np.ascontiguousarray(w2f.transpose(1, 0, 2).reshape(H * E, D)).astype(BF16)
    b1all = np.ascontiguousarray(b1f.T.reshape(E * H))                    # [768]
    b1r = np.ascontiguousarray(b1all.reshape(KC, 128).T).astype(F32)      # [128, 6]
    b2sb = b2f.astype(BF16)
    rb8 = np.zeros((8, 1), dtype=F32)
    rb8[:E, 0] = rb

    ee = np.zeros((E, 128), dtype=BF16)
    for e in range(E):
        ee[e, e::E] = 1

    in_maps = []
    for c in range(N_CORES):
        sl = slice(c * TOK, (c + 1) * TOK)
        in_maps.append(
            {
                "x32": np.ascontiguousarray(xf[sl]),
                "xht": np.ascontiguousarray(x_hi[sl].T),
                "xlt": np.ascontiguousarray(x_lo[sl].T),
                "w1s": w1s,
                "w2s": w2s,
                "rwhl": rwhl,
                "eexp": ee,
                "b2s": b2sb,
                "b1r": b1r,
                "rb8": rb8,
            }
        )
    return in_maps


def _get_nc():
    global _NC_CACHE
    if _NC_CACHE is None:
        _NC_CACHE = _build_bass()
    return _NC_CACHE


def kernel(x, router_w, router_b, w1, b1, w2, b2, _trace=False, _trace_kwargs=None):
    in_maps = _prep_inputs(x, router_w, router_b, w1, b1, w2, b2)
    nc = _get_nc()
    res = run_bass_kernel_spmd(
        nc,
        in_maps,
        core_ids=list(range(N_CORES)),
        trace=_trace,
        **(_trace_kwargs or {}),
    )
    outs = [r["out"] for r in res.results]
    full = np.concatenate(outs, axis=0).reshape(B, S, D)
    if _trace:
        kernel.last_results = res
    return full


# revision 32
# speedup vs baseline: 1.0333x; 1.0333x over previous
"""MoE adapter (top-1 of 4 experts, dense all-expert reference) on 8 TRN2 NeuronCores.

Strategy
--------
Data-parallel over the 32768 tokens (4096 per core); expert weights replicated.

The reference computes every expert's bottleneck MLP (D=768 -> H=192 -> D=768)
on all tokens and combines with the one-hot top-1 dispatch mask.  Since
4 experts x H=192 = 768, the four expert MLPs stack into two dense 768x768
matmuls:

    h_all = gelu(x @ W1_stacked + b1_stacked)       # [T, 768]
    mh    = h_all * expand(one_hot)                  # zero non-selected blocks
    y     = mh @ W2_stacked + one_hot @ b2           # [T, 768]
    out   = y + x

(the mask commutes with gelu because it is 0/1 valued).

On-chip layouts avoid all transposes:
  * host ships x three ways: token-major fp32 (skip add), and feature-major
    bf16 hi/lo split pair (matmul operand + exact-enough router),
  * mm1 produces h feature-major [H, tok] (lhsT = W1 chunk, rhs = x^T),
  * mm2 uses the masked h as the *stationary* operand so y comes out
    token-major [tok, D] and DMAs out contiguously.
  * router logits use the full (x_hi+x_lo) @ (rw_hi+rw_lo) cross product in
    fp32 PSUM accumulation -> bitwise-faithful argmax vs the fp32 reference.
"""

import numpy as np
import ml_dtypes

import concourse.bass as bass
import concourse.mybir as mybir
import concourse.tile as tile
from concourse import bacc
from concourse.bass_utils import run_bass_kernel_spmd

BF16 = ml_dtypes.bfloat16
F32 = np.float32

B, S, D = 16, 2048, 768
H, E = 192, 4
N_CORES = 8
TOK_TOTAL = B * S                 # 32768
TOK = TOK_TOTAL // N_CORES        # 4096 tokens per core
TILE = 512                        # tokens per pipeline tile
N_TILES = TOK // TILE             # 8
SUBT = TILE // 128                # 4 token subtiles of 128
KC = D // 128                     # 6 contraction chunks

_NC_CACHE = None


def _build_bass():
    dt = mybir.dt
    nc = bacc.Bacc("TRN2", target_bir_lowering=False)

    x32 = nc.dram_tensor("x32", [TOK, D], dt.float32, kind="ExternalInput")
    xht = nc.dram_tensor("xht", [D, TOK], dt.bfloat16, kind="ExternalInput")
    xlt = nc.dram_tensor("xlt", [D, TOK], dt.bfloat16, kind="ExternalInput")
    w1s = nc.dram_tensor("w1s", [D, D], dt.bfloat16, kind="ExternalInput")
    w2s = nc.dram_tensor("w2s", [D, D], dt.bfloat16, kind="ExternalInput")
    rwhl = nc.dram_tensor("rwhl", [D, 8], dt.bfloat16, kind="ExternalInput")
    eexp = nc.dram_tensor("eexp", [E, 128], dt.bfloat16, kind="ExternalInput")
    b2s = nc.dram_tensor("b2s", [E, D], dt.bfloat16, kind="ExternalInput")
    b1r = nc.dram_tensor("b1r", [128, KC], dt.float32, kind="ExternalInput")
    rb8 = nc.dram_tensor("rb8", [8, 1], dt.float32, kind="ExternalInput")
    out = nc.dram_tensor("out", [TOK, D], dt.float32, kind="ExternalOutput")

    # feature-major x views: [128 partitions, chunk, token]
    xht_r = xht.rearrange("(c p) t -> p c t", p=128)
    xlt_r = xlt.rearrange("(c p) t -> p c t", p=128)

    add = mybir.AluOpType.add
    mult = mybir.AluOpType.mult
    amax = mybir.AluOpType.max
    iseq = mybir.AluOpType.is_equal

    with tile.TileContext(nc) as tc:
        with (
            tc.tile_pool(name="const", bufs=1) as const,
            tc.tile_pool(name="xin", bufs=3) as xin,
            tc.tile_pool(name="hbuf", bufs=2) as hbuf,
            tc.tile_pool(name="obuf", bufs=2) as obuf,
            tc.tile_pool(name="small", bufs=2) as small,
            tc.tile_pool(name="ps_r", bufs=1, space="PSUM") as ps_r,
            tc.tile_pool(name="ps_h", bufs=2, space="PSUM") as ps_h,
            tc.tile_pool(name="ps_m", bufs=1, space="PSUM") as ps_m,
            tc.tile_pool(name="ps_y", bufs=4, space="PSUM") as ps_y,
        ):
            # Small constants go on the gpsimd (SWDGE) queue; the two big weight
            # matrices ride the sync HWDGE FIFO interleaved with tile-0's loads
            # in exact first-use order (xh, xl, w1, x32, w2).
            rwsb = const.tile([128, KC, 8], dt.bfloat16)
            nc.gpsimd.dma_start(rwsb, rwhl.rearrange("(c p) e -> p c e", p=128))
            rbsb = const.tile([8, 1], dt.float32)
            nc.gpsimd.dma_start(rbsb, rb8[:])
            b1sb = const.tile([128, KC], dt.float32)
            nc.gpsimd.dma_start(b1sb, b1r[:])
            eesb = const.tile([E, 128], dt.bfloat16)
            nc.gpsimd.dma_start(eesb, eexp[:])
            b2sb = const.tile([E, D], dt.bfloat16)
            nc.gpsimd.dma_start(b2sb, b2s[:])
            w1sb = const.tile([128, KC, D], dt.bfloat16)
            w2sb = const.tile([128, KC, D], dt.bfloat16)

            def load_tiles(it):
                t0 = it * TILE
                xh = xin.tile([128, KC, TILE], dt.bfloat16, tag="xh")
                nc.sync.dma_start(xh, xht_r[:, :, t0 : t0 + TILE])
                xl = xin.tile([128, KC, TILE], dt.bfloat16, tag="xl")
                nc.sync.dma_start(xl, xlt_r[:, :, t0 : t0 + TILE])
                if it == 0:
                    nc.sync.dma_start(
                        w1sb, w1s.rearrange("(c p) h -> p c h", p=128)
                    )
                x32t = xin.tile([128, SUBT, D], dt.float32, tag="x32t")
                nc.sync.dma_start(
                    x32t, x32[t0 : t0 + TILE].rearrange("(a p) d -> p a d", p=128)
                )
                if it == 0:
                    nc.sync.dma_start(
                        w2sb, w2s.rearrange("(c p) h -> p c h", p=128)
                    )
                return xh, xl, x32t

            def router_onehot(xh, xl):
                """logits^T in psum -> one-hot mask mt32[0:4] [4, TILE] bf16."""
                # rows 0:4 accumulate (x_hi + x_lo) @ rw_hi  (+rb via ACT bias)
                # rows 4:8 accumulate (x_hi + x_lo) @ rw_lo
                psr = ps_r.tile([8, TILE], dt.float32, tag="psr")
                for kc in range(KC):
                    nc.tensor.matmul(
                        psr, rwsb[:, kc, :], xh[:, kc, :],
                        start=(kc == 0), stop=False,
                    )
                for kc in range(KC):
                    nc.tensor.matmul(
                        psr, rwsb[:, kc, :], xl[:, kc, :],
                        start=False, stop=(kc == KC - 1),
                    )
                # alignment-safe one-hot argmax via DVE 32x32 stream transpose
                lt32s = small.tile([32, TILE], dt.float32, tag="lt32s")
                nc.scalar.activation(
                    lt32s[0:8], psr,
                    mybir.ActivationFunctionType.Identity,
                    bias=rbsb, scale=1.0,
                )
                # token-major blocks: lt32[p, 32g+r] = lt32s[r, 32g+p]
                lt32 = small.tile([32, TILE], dt.float32, tag="lt32")
                nc.vector.transpose(lt32, lt32s)
                v = lt32.rearrange("p (g r) -> p g r", r=32)
                lt_tok = small.tile([32, TILE // 32, E], dt.float32, tag="lt_tok")
                nc.vector.tensor_tensor(lt_tok, v[:, :, 0:4], v[:, :, 4:8], add)
                mxg = small.tile([32, TILE // 32], dt.float32, tag="mxg")
                nc.vector.tensor_reduce(
                    out=mxg, in_=lt_tok, axis=mybir.AxisListType.X, op=amax
                )
                mtb = small.tile([32, TILE], dt.bfloat16, tag="mtb")
                mview = mtb.rearrange("p (g r) -> p g r", r=32)
                nc.vector.tensor_tensor(
                    mview[:, :, 0:4], lt_tok,
                    mxg[:, :, None].to_broadcast((32, TILE // 32, E)), iseq,
                )
                # back-transpose: mt32[e, t] = one_hot[t, e] for e < 4
                mt32 = small.tile([32, TILE], dt.bfloat16, tag="mt32")
                nc.vector.transpose(mt32, mtb)
                return mt32

            # ---- PE warm-up burst: spin the HAM up to K=8/8 during the DMA head
            dummy = const.tile([128, TILE], dt.bfloat16)
            nc.vector.memset(dummy, 0.0)
            psd = ps_h.tile([128, TILE], dt.float32, tag="psh")
            for _ in range(10):
                nc.tensor.matmul(psd, dummy[:, 0:128], dummy, start=True, stop=True)

            # software pipeline: router/one-hot for tile n+1 issues at the end of
            # iteration n, so the mask chain latency hides under mm1/mm2.
            # Loads run two tiles ahead.
            tiles = {0: load_tiles(0)}
            mt32 = router_onehot(tiles[0][0], tiles[0][1])
            tiles[1] = load_tiles(1)

            for it in range(N_TILES):
                t0 = it * TILE
                mt = mt32[0:4]
                xh, xl, x32t = tiles[it]

                if it + 2 < N_TILES:
                    tiles[it + 2] = load_tiles(it + 2)

                # ---- mm1: h^T = gelu(W1^T x + b1), then mask ----
                # experts are interleaved along H (unit j of expert e at 4j+e),
                # so the expanded one-hot is the same [128, TILE] tile for every
                # H-chunk: a single K=4 matmul per tile.
                psm = ps_m.tile([128, TILE], dt.float32, tag="psm")
                nc.tensor.matmul(psm, eesb, mt, start=True, stop=True)
                mh = hbuf.tile([128, KC, TILE], dt.bfloat16, tag="mh")
                hchunk = hbuf.tile([128, KC, TILE], dt.bfloat16, tag="hchunk")
                for hc in range(KC):
                    psh = ps_h.tile([128, TILE], dt.float32, tag="psh")
                    for kc in range(KC):
                        nc.tensor.matmul(
                            psh,
                            w1sb[:, kc, hc * 128 : (hc + 1) * 128],
                            xh[:, kc, :],
                            start=(kc == 0), stop=(kc == KC - 1),
                        )
                    nc.scalar.activation(
                        hchunk[:, hc, :], psh,
                        mybir.ActivationFunctionType.Gelu,
                        bias=b1sb[:, hc : hc + 1], scale=1.0,
                    )
                    nc.vector.tensor_tensor(
                        mh[:, hc, :], hchunk[:, hc, :], psm, mult
                    )

                # ---- mm2: y = mh^T.T @ W2 + one_hot @ b2, token-major ----
                out_r = out[t0 : t0 + TILE].rearrange("(a p) d -> p a d", p=128)
                for a in range(SUBT):
                    osb = obuf.tile([128, D], dt.float32, tag="osb")
                    for half in range(2):
                        d0 = half * 384
                        psy = ps_y.tile([128, 384], dt.float32, tag="psy")
                        for hc in range(KC):
                            nc.tensor.matmul(
                                psy,
                                mh[:, hc, a * 128 : (a + 1) * 128],
                                w2sb[:, hc, d0 : d0 + 384],
                                start=(hc == 0), stop=False,
                            )
                        nc.tensor.matmul(
                            psy,
                            mt[:, a * 128 : (a + 1) * 128],
                            b2sb[:, d0 : d0 + 384],
                            start=False, stop=True,
                        )
                        nc.vector.tensor_tensor(
                            osb[:, d0 : d0 + 384], psy,
                            x32t[:, a, d0 : d0 + 384], add,
                        )
                    # per-subtile store on the ACT HWDGE ring (doesn't block loads)
                    nc.scalar.dma_start(out_r[:, a, :], osb)

                if it + 1 < N_TILES:
                    mt32 = router_onehot(tiles[it + 1][0], tiles[it + 1][1])
                del tiles[it]

    nc.compile()
    return nc


def _prep_inputs(x, router_w, router_b, w1, b1, w2, b2):
    """Host-side packing: split/cast/transpose; returns per-core input dicts."""
    xf = np.ascontiguousarray(np.asarray(x, dtype=F32).reshape(TOK_TOTAL, D))
    x_hi = xf.astype(BF16)
    x_lo = (xf - x_hi.astype(F32)).astype(BF16)

    rw = np.asarray(router_w, dtype=F32)
    rw_hi = rw.astype(BF16)
    rw_lo = (rw - rw_hi.astype(F32)).astype(BF16)
    rwhl = np.ascontiguousarray(np.concatenate([rw_hi, rw_lo], axis=1))  # [D, 8]

    w1f = np.asarray(w1, dtype=F32)           # [E, D, H]
    w2f = np.asarray(w2, dtype=F32)           # [E, H, D]
    b1f = np.asarray(b1, dtype=F32)           # [E, H]
    b2f = np.asarray(b2, dtype=F32)           # [E, D]
    rb = np.asarray(router_b, dtype=F32)      # [E]

    # experts interleaved along the stacked hidden dim: unit j of expert e
    # lives at index 4j + e  -> the one-hot expansion pattern repeats every
    # 4 partitions, identically for each 128-row chunk.
    w1s = np.ascontiguousarray(w1f.transpose(1, 2, 0).reshape(D, H * E)).astype(BF16)
    w2s = np.ascontiguousarray(w2f.transpose(1, 0, 2).reshape(H * E, D)).astype(BF16)
    b1all = np.ascontiguousarray(b1f.T.reshape(E * H))                    # [768]
    b1r = np.ascontiguousarray(b1all.reshape(KC, 128).T).astype(F32)      # [128, 6]
    b2sb = b2f.astype(BF16)
    rb8 = np.zeros((8, 1), dtype=F32)
    rb8[:E, 0] = rb

    ee = np.zeros((E, 128), dtype=BF16)
    for e in range(E):
        ee[e, e::E] = 1

    in_maps = []
    for c in range(N_CORES):
        sl = slice(c * TOK, (c + 1) * TOK)
        in_maps.append(
            {
                "x32": np.ascontiguousarray(xf[sl]),
                "xht": np.ascontiguousarray(x_hi[sl].T),
                "xlt": np.ascontiguousarray(x_lo[sl].T),
                "w1s": w1s,
                "w2s": w2s,
                "rwhl": rwhl,
                "eexp": ee,
                "b2s": b2sb,
                "b1r": b1r,
                "rb8": rb8,
            }
        )
    return in_maps


def _get_nc():
    global _NC_CACHE
    if _NC_CACHE is None:
        _NC_CACHE = _build_bass()
    return _NC_CACHE


def kernel(x, router_w, router_b, w1, b1, w2, b2, _trace=False, _trace_kwargs=None):
    in_maps = _prep_inputs(x, router_w, router_b, w1, b1, w2, b2)
    nc = _get_nc()
    res = run_bass_kernel_spmd(
        nc,
        in_maps,
        core_ids=list(range(N_CORES)),
        trace=_trace,
        **(_trace_kwargs or {}),
    )
    outs = [r["out"] for r in res.results]
    full = np.concatenate(outs, axis=0).reshape(B, S, D)
    if _trace:
        kernel.last_results = res
    return full


# revision 34
# speedup vs baseline: 1.0466x; 1.0129x over previous
"""MoE adapter (top-1 of 4 experts, dense all-expert reference) on 8 TRN2 NeuronCores.

Strategy
--------
Data-parallel over the 32768 tokens (4096 per core); expert weights replicated.

The reference computes every expert's bottleneck MLP (D=768 -> H=192 -> D=768)
on all tokens and combines with the one-hot top-1 dispatch mask.  Since
4 experts x H=192 = 768, the four expert MLPs stack into two dense 768x768
matmuls:

    h_all = gelu(x @ W1_stacked + b1_stacked)       # [T, 768]
    mh    = h_all * expand(one_hot)                  # zero non-selected blocks
    y     = mh @ W2_stacked + one_hot @ b2           # [T, 768]
    out   = y + x

(the mask commutes with gelu because it is 0/1 valued).

On-chip layouts avoid all transposes:
  * host ships x three ways: token-major fp32 (skip add), and feature-major
    bf16 hi/lo split pair (matmul operand + exact-enough router),
  * mm1 produces h feature-major [H, tok] (lhsT = W1 chunk, rhs = x^T),
  * mm2 uses the masked h as the *stationary* operand so y comes out
    token-major [tok, D] and DMAs out contiguously.
  * router logits use the full (x_hi+x_lo) @ (rw_hi+rw_lo) cross product in
    fp32 PSUM accumulation -> bitwise-faithful argmax vs the fp32 reference.
"""

import numpy as np
import ml_dtypes

import concourse.bass as bass
import concourse.mybir as mybir
import concourse.tile as tile
from concourse import bacc
from concourse.bass_utils import run_bass_kernel_spmd

BF16 = ml_dtypes.bfloat16
F32 = np.float32

B, S, D = 16, 2048, 768
H, E = 192, 4
N_CORES = 8
TOK_TOTAL = B * S                 # 32768
TOK = TOK_TOTAL // N_CORES        # 4096 tokens per core
TILE = 512                        # tokens per pipeline tile
N_TILES = TOK // TILE             # 8
SUBT = TILE // 128                # 4 token subtiles of 128
KC = D // 128                     # 6 contraction chunks

_NC_CACHE = None


def _build_bass():
    dt = mybir.dt
    nc = bacc.Bacc("TRN2", target_bir_lowering=False)

    x32 = nc.dram_tensor("x32", [TOK, D], dt.float32, kind="ExternalInput")
    xht = nc.dram_tensor("xht", [D, TOK], dt.bfloat16, kind="ExternalInput")
    xlt = nc.dram_tensor("xlt", [D, TOK], dt.bfloat16, kind="ExternalInput")
    w1s = nc.dram_tensor("w1s", [D, D], dt.bfloat16, kind="ExternalInput")
    w2s = nc.dram_tensor("w2s", [D, D], dt.bfloat16, kind="ExternalInput")
    rwhl = nc.dram_tensor("rwhl", [D, 8], dt.bfloat16, kind="ExternalInput")
    eexp = nc.dram_tensor("eexp", [E, 128], dt.bfloat16, kind="ExternalInput")
    b2s = nc.dram_tensor("b2s", [E, D], dt.bfloat16, kind="ExternalInput")
    b1r = nc.dram_tensor("b1r", [128, KC], dt.float32, kind="ExternalInput")
    rb8 = nc.dram_tensor("rb8", [8, 1], dt.float32, kind="ExternalInput")
    out = nc.dram_tensor("out", [TOK, D], dt.float32, kind="ExternalOutput")

    # feature-major x views: [128 partitions, chunk, token]
    xht_r = xht.rearrange("(c p) t -> p c t", p=128)
    xlt_r = xlt.rearrange("(c p) t -> p c t", p=128)

    add = mybir.AluOpType.add
    mult = mybir.AluOpType.mult
    amax = mybir.AluOpType.max
    iseq = mybir.AluOpType.is_equal

    with tile.TileContext(nc) as tc:
        with (
            tc.tile_pool(name="const", bufs=1) as const,
            tc.tile_pool(name="xin", bufs=3) as xin,
            tc.tile_pool(name="hbuf", bufs=3) as hbuf,
            tc.tile_pool(name="obuf", bufs=4) as obuf,
            tc.tile_pool(name="small", bufs=3) as small,
            tc.tile_pool(name="ps_r", bufs=1, space="PSUM") as ps_r,
            tc.tile_pool(name="ps_h", bufs=2, space="PSUM") as ps_h,
            tc.tile_pool(name="ps_m", bufs=1, space="PSUM") as ps_m,
            tc.tile_pool(name="ps_y", bufs=4, space="PSUM") as ps_y,
        ):
            # Small constants go on the gpsimd (SWDGE) queue; the two big weight
            # matrices ride the sync HWDGE FIFO interleaved with tile-0's loads
            # in exact first-use order (xh, xl, w1, x32, w2).
            rwsb = const.tile([128, KC, 8], dt.bfloat16)
            nc.gpsimd.dma_start(rwsb, rwhl.rearrange("(c p) e -> p c e", p=128))
            rbsb = const.tile([8, 1], dt.float32)
            nc.gpsimd.dma_start(rbsb, rb8[:])
            b1sb = const.tile([128, KC], dt.float32)
            nc.gpsimd.dma_start(b1sb, b1r[:])
            eesb = const.tile([E, 128], dt.bfloat16)
            nc.gpsimd.dma_start(eesb, eexp[:])
            b2sb = const.tile([E, D], dt.bfloat16)
            nc.gpsimd.dma_start(b2sb, b2s[:])
            w1sb = const.tile([128, KC, D], dt.bfloat16)
            w2sb = const.tile([128, KC, D], dt.bfloat16)

            def load_tiles(it):
                t0 = it * TILE
                xh = xin.tile([128, KC, TILE], dt.bfloat16, tag="xh")
                nc.sync.dma_start(xh, xht_r[:, :, t0 : t0 + TILE])
                xl = xin.tile([128, KC, TILE], dt.bfloat16, tag="xl")
                nc.sync.dma_start(xl, xlt_r[:, :, t0 : t0 + TILE])
                if it == 0:
                    nc.sync.dma_start(
                        w1sb, w1s.rearrange("(c p) h -> p c h", p=128)
                    )
                x32t = xin.tile([128, SUBT, D], dt.float32, tag="x32t")
                nc.sync.dma_start(
                    x32t, x32[t0 : t0 + TILE].rearrange("(a p) d -> p a d", p=128)
                )
                if it == 0:
                    nc.sync.dma_start(
                        w2sb, w2s.rearrange("(c p) h -> p c h", p=128)
                    )
                return xh, xl, x32t

            def router_onehot(xh, xl, between=None):
                """logits^T in psum -> one-hot mask mt32[0:4] [4, TILE] bf16.

                `between`, if given, is traced between the hi and lo matmul
                groups (tile-0 ramp: lets PE work while xl is still loading).
                """
                # rows 0:4 accumulate (x_hi + x_lo) @ rw_hi  (+rb via ACT bias)
                # rows 4:8 accumulate (x_hi + x_lo) @ rw_lo
                psr = ps_r.tile([8, TILE], dt.float32, tag="psr")
                for kc in range(KC):
                    nc.tensor.matmul(
                        psr, rwsb[:, kc, :], xh[:, kc, :],
                        start=(kc == 0), stop=False,
                    )
                if between is not None:
                    between()
                for kc in range(KC):
                    nc.tensor.matmul(
                        psr, rwsb[:, kc, :], xl[:, kc, :],
                        start=False, stop=(kc == KC - 1),
                    )
                # alignment-safe one-hot argmax via DVE 32x32 stream transpose
                lt32s = small.tile([32, TILE], dt.float32, tag="lt32s")
                nc.scalar.activation(
                    lt32s[0:8], psr,
                    mybir.ActivationFunctionType.Identity,
                    bias=rbsb, scale=1.0,
                )
                # token-major blocks: lt32[p, 32g+r] = lt32s[r, 32g+p]
                lt32 = small.tile([32, TILE], dt.float32, tag="lt32")
                nc.vector.transpose(lt32, lt32s)
                v = lt32.rearrange("p (g r) -> p g r", r=32)
                lt_tok = small.tile([32, TILE // 32, E], dt.float32, tag="lt_tok")
                nc.vector.tensor_tensor(lt_tok, v[:, :, 0:4], v[:, :, 4:8], add)
                mxg = small.tile([32, TILE // 32], dt.float32, tag="mxg")
                nc.vector.tensor_reduce(
                    out=mxg, in_=lt_tok, axis=mybir.AxisListType.X, op=amax
                )
                mtb = small.tile([32, TILE], dt.bfloat16, tag="mtb")
                mview = mtb.rearrange("p (g r) -> p g r", r=32)
                nc.vector.tensor_tensor(
                    mview[:, :, 0:4], lt_tok,
                    mxg[:, :, None].to_broadcast((32, TILE // 32, E)), iseq,
                )
                # back-transpose: mt32[e, t] = one_hot[t, e] for e < 4
                mt32 = small.tile([32, TILE], dt.bfloat16, tag="mt32")
                nc.vector.transpose(mt32, mtb)
                return mt32

            # ---- PE warm-up burst: spin the HAM up to K=8/8 during the DMA head
            dummy = const.tile([128, TILE], dt.bfloat16)
            nc.vector.memset(dummy, 0.0)
            psd = ps_h.tile([128, TILE], dt.float32, tag="psh")
            for _ in range(10):
                nc.tensor.matmul(psd, dummy[:, 0:128], dummy, start=True, stop=True)

            # software pipeline: router/one-hot for tile n+1 issues at the end of
            # iteration n, so the mask chain latency hides under mm1/mm2.
            # Loads run two tiles ahead.
            tiles = {0: load_tiles(0)}
            mt32 = router_onehot(tiles[0][0], tiles[0][1])
            tiles[1] = load_tiles(1)

            for it in range(N_TILES):
                t0 = it * TILE
                mt = mt32[0:4]
                xh, xl, x32t = tiles[it]

                if it + 2 < N_TILES:
                    tiles[it + 2] = load_tiles(it + 2)

                # ---- mm1: h^T = gelu(W1^T x + b1), then mask ----
                # experts are interleaved along H (unit j of expert e at 4j+e),
                # so the expanded one-hot is the same [128, TILE] tile for every
                # H-chunk: a single K=4 matmul per tile.
                psm = ps_m.tile([128, TILE], dt.float32, tag="psm")
                nc.tensor.matmul(psm, eesb, mt, start=True, stop=True)
                mh = hbuf.tile([128, KC, TILE], dt.bfloat16, tag="mh")
                hchunk = hbuf.tile([128, KC, TILE], dt.bfloat16, tag="hchunk")
                for hc in range(KC):
                    psh = ps_h.tile([128, TILE], dt.float32, tag="psh")
                    for kc in range(KC):
                        nc.tensor.matmul(
                            psh,
                            w1sb[:, kc, hc * 128 : (hc + 1) * 128],
                            xh[:, kc, :],
                            start=(kc == 0), stop=(kc == KC - 1),
                        )
                    nc.scalar.activation(
                        hchunk[:, hc, :], psh,
                        mybir.ActivationFunctionType.Gelu,
                        bias=b1sb[:, hc : hc + 1], scale=1.0,
                    )
                    nc.vector.tensor_tensor(
                        mh[:, hc, :], hchunk[:, hc, :], psm, mult
                    )

                # ---- mm2: y = mh^T.T @ W2 + one_hot @ b2, token-major ----
                out_r = out[t0 : t0 + TILE].rearrange("(a p) d -> p a d", p=128)
                for a in range(SUBT):
                    osb = obuf.tile([128, D], dt.float32, tag="osb")
                    for half in range(2):
                        d0 = half * 384
                        psy = ps_y.tile([128, 384], dt.float32, tag="psy")
                        for hc in range(KC):
                            nc.tensor.matmul(
                                psy,
                                mh[:, hc, a * 128 : (a + 1) * 128],
                                w2sb[:, hc, d0 : d0 + 384],
                                start=(hc == 0), stop=False,
                            )
                        nc.tensor.matmul(
                            psy,
                            mt[:, a * 128 : (a + 1) * 128],
                            b2sb[:, d0 : d0 + 384],
                            start=False, stop=True,
                        )
                        nc.vector.tensor_tensor(
                            osb[:, d0 : d0 + 384], psy,
                            x32t[:, a, d0 : d0 + 384], add,
                        )
                    # per-subtile store on the ACT HWDGE ring (doesn't block loads)
                    nc.scalar.dma_start(out_r[:, a, :], osb)

                if it + 1 < N_TILES:
                    mt32 = router_onehot(tiles[it + 1][0], tiles[it + 1][1])
                del tiles[it]

    nc.compile()
    return nc


def _prep_inputs(x, router_w, router_b, w1, b1, w2, b2):
    """Host-side packing: split/cast/transpose; returns per-core input dicts."""
    xf = np.ascontiguousarray(np.asarray(x, dtype=F32).reshape(TOK_TOTAL, D))
    x_hi = xf.astype(BF16)
    x_lo = (xf - x_hi.astype(F32)).astype(BF16)

    rw = np.asarray(router_w, dtype=F32)
    rw_hi = rw.astype(BF16)
    rw_lo = (rw - rw_hi.astype(F32)).astype(BF16)
    rwhl = np.ascontiguousarray(np.concatenate([rw_hi, rw_lo], axis=1))  # [D, 8]

    w1f = np.asarray(w1, dtype=F32)           # [E, D, H]
    w2f = np.asarray(w2, dtype=F32)           # [E, H, D]
    b1f = np.asarray(b1, dtype=F32)           # [E, H]
    b2f = np.asarray(b2, dtype=F32)           # [E, D]
    rb = np.asarray(router_b, dtype=F32)      # [E]

    # experts interleaved along the stacked hidden dim: unit j of expert e
    # lives at index 4j + e  -> the one-hot expansion pattern repeats every
    # 4 partitions, identically for each 128-row chunk.
    w1s = np.ascontiguousarray(w1f.transpose(1, 2, 0).reshape(D, H * E)).astype(BF16)
    w2s = np.ascontiguousarray(w2f.transpose(1, 0, 2).reshape(H * E, D)).astype(BF16)
    b1all = np.ascontiguousarray(b1f.T.reshape(E * H))                    # [768]
    b1r = np.ascontiguousarray(b1all.reshape(KC, 128).T).astype(F32)      # [128, 6]
    b2sb = b2f.astype(BF16)
    rb8 = np.zeros((8, 1), dtype=F32)
    rb8[:E, 0] = rb

    ee = np.zeros((E, 128), dtype=BF16)
    for e in range(E):
        ee[e, e::E] = 1

    in_maps = []
    for c in range(N_CORES):
        sl = slice(c * TOK, (c + 1) * TOK)
        in_maps.append(
            {
                "x32": np.ascontiguousarray(xf[sl]),
                "xht": np.ascontiguousarray(x_hi[sl].T),
                "xlt": np.ascontiguousarray(x_lo[sl].T),
                "w1s": w1s,
                "w2s": w2s,
                "rwhl": rwhl,
                "eexp": ee,
                "b2s": b2sb,
                "b1r": b1r,
                "rb8": rb8,
            }
        )
    return in_maps


def _get_nc():
    global _NC_CACHE
    if _NC_CACHE is None:
        _NC_CACHE = _build_bass()
    return _NC_CACHE


def kernel(x, router_w, router_b, w1, b1, w2, b2, _trace=False, _trace_kwargs=None):
    in_maps = _prep_inputs(x, router_w, router_b, w1, b1, w2, b2)
    nc = _get_nc()
    res = run_bass_kernel_spmd(
        nc,
        in_maps,
        core_ids=list(range(N_CORES)),
        trace=_trace,
        **(_trace_kwargs or {}),
    )
    outs = [r["out"] for r in res.results]
    full = np.concatenate(outs, axis=0).reshape(B, S, D)
    if _trace:
        kernel.last_results = res
    return full
